# revision 46
# baseline (speedup 1.0000x reference)
"""Multi-head self-attention (B=2, S=2048, D=1024, H=16, causal) on 8 TRN2 cores.

Sharding: core c handles batch b=c//4 and head-group g=c%4 (4 heads each).
Host pre-transposes/pre-chunks everything into DMA-friendly layouts and
converts matmul operands to bf16 (fp32 psum accumulation keeps rel-err ~5e-3):
  xc   [4, 128, 8, 512]  bf16: xc[cb,p,ko,s] = x[b][cb*512+s, ko*128+p]
  wq/wk/wv [128, 8, 256] bf16: w[p,ko,m] = W.T[ko*128+p, g*256+m]
  wo   [128, 2, 1024]    bf16: wo[p,co,n] = Wo[n, g*256+co*128+p]
  mask [128, 2, 128]     bf16 lower-triangular (p <= i)
Host sums the 4 per-group partial outputs per batch at the end.

On-chip dataflow per core:
  qT/kT [128, 512] per (head-pair mo, 512-query chunk cb), head dim on
  partitions; v [128, 4*65] per 128-key chunk (ones column per head so the
  PV matmul accumulates the softmax denominator in psum row 64).  Scores are
  computed transposed (scoresT[j, i]) so softmax needs no transpose; no
  max-subtraction (scores are O(+-8), exp is safe in fp32 psum).

Schedule (the point of this version): projections are pipelined against the
x DMA stream per 512-query chunk; attention chunks are software-pipelined
(scores of chunk jc+1 issue before PV of jc so the scalar-engine exp latency
is hidden); remaining projection chunks and deferred output projections are
injected between attention chunks so the PE never idles long enough for the
HAM clock-gate to re-throttle it to 1.2 GHz.  The softmax normalization runs
entirely off the PE: vector reciprocal -> gpsimd partition_broadcast ->
one scalar_tensor_tensor reading psum directly.  Diagonal chunks skip their
fully-masked columns.
"""

import os
import sys
from collections import deque

sys.path.insert(0, "/opt/trn_rl_repo")
os.environ.setdefault("MYCRO_LOCAL_CACHE", "1")

import numpy as np
import ml_dtypes

import concourse.bacc as bacc
import concourse.bass as bass
import concourse.mybir as mybir
import concourse.tile as tile
from concourse import bass_utils

# The agent image's antenv lacks axon_hooks, so bass_utils' trace path dies on
# import.  Register a shim module that lazily builds the ctypes NTFF hook.
if "antenv.axon_hooks" not in sys.modules:
    import types

    _shim = types.ModuleType("antenv.axon_hooks")
    _shim._HOOK = None

    def _set_hook(hook, _m=_shim):
        _m._HOOK = hook

    def _get_hook(_m=_shim):
        if _m._HOOK is None:
            try:
                from trn_agent_boot.trn_boot import _ntff_profile_via_ctypes

                _m._HOOK = _ntff_profile_via_ctypes("/opt/axon/libaxon_pjrt.so")
            except Exception:
                _m._HOOK = None
        return _m._HOOK

    _shim.set_axon_ntff_profile_hook = _set_hook
    _shim.get_axon_ntff_profile_hook = _get_hook
    sys.modules["antenv.axon_hooks"] = _shim

B, S, D, H = 2, 2048, 1024, 16
DK = 64                      # head dim
HC = 4                       # heads per core
GC = HC * DK                 # 256 cols per head-group
N_CORES = 8
SCALE = 1.0 / np.sqrt(DK)    # 0.125

F32 = mybir.dt.float32
BF16 = mybir.dt.bfloat16
MULT = mybir.AluOpType.mult

TRACE = False
LAST_RESULTS = None


def build_bass():
    nc = bacc.Bacc("TRN2", target_bir_lowering=False, debug=False)

    x_d = nc.dram_tensor("xc", [4, 128, 8, 512], BF16, kind="ExternalInput")
    wq_d = nc.dram_tensor("wq", [128, 8, GC], BF16, kind="ExternalInput")
    wk_d = nc.dram_tensor("wk", [128, 8, GC], BF16, kind="ExternalInput")
    wv_d = nc.dram_tensor("wv", [128, 8, GC], BF16, kind="ExternalInput")
    wo_d = nc.dram_tensor("wo", [128, 2, D], BF16, kind="ExternalInput")
    mask_d = nc.dram_tensor("mask", [128, 2, 128], BF16, kind="ExternalInput")
    out_d = nc.dram_tensor("out", [16, 128, D], BF16, kind="ExternalOutput")

    EXP = mybir.ActivationFunctionType.Exp

    with tile.TileContext(nc) as tc:
        with (
            nc.allow_low_precision(reason="bf16 matmuls, fp32 psum; ~5e-3 rel err"),
            tc.tile_pool(name="const", bufs=1) as const,
            tc.tile_pool(name="work", bufs=3) as work,
            tc.tile_pool(name="apool", bufs=2) as apool,
            tc.tile_pool(name="opool", bufs=2) as opool,
            tc.tile_pool(name="rpool", bufs=2) as rpool,
            tc.tile_pool(name="psmm", bufs=2, space="PSUM") as psmm,
            tc.tile_pool(name="psout", bufs=4, space="PSUM") as psout,
        ):
            # ---- input DMA.  Only what the first projection chunk needs is
            # issued up front (wq+x0 on sync, wk/wv/mask/wo on gpsimd) so the
            # first-needed transfers get the full HBM bandwidth; x1-x3 are
            # issued from the vector queue behind the cb0 psum casts, by which
            # time the early transfers have drained.
            xts = []
            for cb in range(4):
                xt = const.tile([128, 8, 512], BF16, name=f"xt{cb}")
                xts.append(xt)
            wq = const.tile([128, 8, GC], BF16)
            wk = const.tile([128, 8, GC], BF16)
            wv = const.tile([128, 8, GC], BF16)
            wo = const.tile([128, 2, D], BF16)
            maskt = const.tile([128, 2, 128], BF16)
            # halves: the ko 0..3 slice lands ~1.4us earlier and the first
            # projection matmuls only need it (subtile deps)
            nc.sync.dma_start(xts[0][:, 0:4, :], x_d[0][:, 0:4, :])
            nc.sync.dma_start(xts[0][:, 4:8, :], x_d[0][:, 4:8, :])
            nc.gpsimd.dma_start(wq[:], wq_d[:])
            nc.gpsimd.dma_start(wk[:], wk_d[:])

            # dummy matmuls with no data dependencies: they run during the
            # initial DMA wait and warm the PE's HAM clock-gate (1.2 -> 2.4
            # GHz needs ~3.4us of sustained activity) so the real projection
            # matmuls start at full clock.
            dmy = const.tile([128, 256], BF16)
            nc.vector.memset(dmy[:], 0.0)

            def emit_warm(n):
                # one psum tile for all n matmuls so only one rotation slot
                # is claimed (a slot per dummy would WAR-chain with real work)
                dps = psmm.tile([128, 256], F32, tag="mm", name="dps")
                for w in range(n):
                    nc.tensor.matmul(dps[:], dmy[:, 0:128], dmy[:],
                                     start=True, stop=True,
                                     skip_group_check=True)

            emit_warm(26)

            # pre-load the exp activation table (otherwise its ~1.3us
            # ACT_TABLE_LOAD lands on the first attention chunk)
            wrm = const.tile([1, 8], F32)
            nc.vector.memset(wrm[:], 0.0)
            wrm2 = const.tile([1, 8], F32)
            nc.scalar.activation(wrm2[:], wrm[:],
                                 mybir.ActivationFunctionType.Exp, scale=1.0)

            # qT/kT per (head-pair mo, 512-col chunk cb); v per 128-key chunk
            qts = [[const.tile([128, 512], BF16, name=f"q{mo}{cb}")
                    for cb in range(4)] for mo in range(2)]
            kts = [[const.tile([128, 512], BF16, name=f"k{mo}{cb}")
                    for cb in range(4)] for mo in range(2)]
            vts = []
            for io in range(16):
                vt = const.tile([128, HC * 65], BF16, name=f"v{io}")
                # ones in every column; V-proj overwrites the 64 value cols,
                # leaving col 64 of each head = 1.0 (denominator row)
                nc.vector.memset(vt[:], 1.0)
                vts.append(vt)
            ones64 = const.tile([1, 64], BF16)
            nc.vector.memset(ones64[:], 1.0)

            # ---- projection emission units --------------------------------
            # psum->sbuf casts must run on the vector engine (gpsimd has no
            # psum access; scalar is saturated by exp)
            def gate_dma(gate_ps, after):
                # real WAW gating: the scalar copy writes one corner of each
                # DMA's destination from the gating psum, so the desc-gen has
                # a hard dependency and the Tile scheduler cannot hoist it.
                # Later transfers then start only once the earlier ones have
                # drained, keeping the first-needed data bandwidth-unstarved.
                for dst_tile, dst_ap, src_ap in after:
                    nc.scalar.copy(dst_tile[0:1, 0:1, 0:1], gate_ps[0:1, 0:1])
                    nc.scalar.dma_start(dst_ap, src_ap)

            def emit_qk_mo(cb, w_sb, dst, mo, after=()):
                ps = psmm.tile([128, 512], F32, tag="mm", name="psqk")
                for ko in range(8):
                    nc.tensor.matmul(
                        ps[:],
                        w_sb[:, ko, mo * 128:(mo + 1) * 128],
                        xts[cb][:, ko, :],
                        start=(ko == 0),
                        stop=(ko == 7),
                        skip_group_check=True,
                    )
                nc.vector.tensor_copy(dst[mo][cb][:], ps[:])
                if after:
                    gate_dma(ps[:], after)

            def emit_v_quarter(cb, io2):
                io = cb * 4 + io2
                ps = psmm.tile([128, 256], F32, tag="mm", name="psv")
                for ko in range(8):
                    nc.tensor.matmul(
                        ps[:],
                        xts[cb][:, ko, io2 * 128:(io2 + 1) * 128],
                        wv[:, ko, :],
                        start=(ko == 0),
                        stop=(ko == 7),
                        skip_group_check=True,
                    )
                nc.vector.tensor_copy(
                    vts[io].rearrange("p (h u) -> p h u", u=65)[:, :, 0:64],
                    ps.rearrange("p (h e) -> p h e", e=64),
                )

            def qk_thunks(cb, w_sb, dst, afters=((), ())):
                return [lambda mo=mo: emit_qk_mo(cb, w_sb, dst, mo, afters[mo])
                        for mo in range(2)]

            def proj_thunks(cb):
                return (
                    qk_thunks(cb, wq, qts)
                    + qk_thunks(cb, wk, kts)
                    + [lambda io2=io2: emit_v_quarter(cb, io2) for io2 in range(4)]
                )

            def emit_proj0():
                for th in qk_thunks(0, wq, qts, afters=(
                    [(wv, wv[:], wv_d[:])],
                    [(xts[1], xts[1][:], x_d[1])],
                )):
                    th()
                for th in qk_thunks(0, wk, kts, afters=(
                    [(maskt, maskt[:], mask_d[:]), (xts[2], xts[2][:], x_d[2])],
                    [(xts[3], xts[3][:], x_d[3]), (wo, wo[:], wo_d[:])],
                )):
                    th()
                for io2 in range(4):
                    emit_v_quarter(0, io2)

            # ---- output projection for one 512-query block ----------------
            aTs = {}

            def emit_po_co(po, aT, co, so, stop):
                for nt in range(2):
                    nc.tensor.matmul(
                        po[:, nt, :],
                        aT[:, co, so * 128:(so + 1) * 128],
                        wo[:, co, nt * 512:(nt + 1) * 512],
                        start=(co == 0),
                        stop=stop,
                        skip_group_check=True,
                    )

            def emit_osb(Q, so, po, engine):
                osb = opool.tile([128, D], BF16, tag="osb", name="osb")
                if engine == "scalar":
                    nc.scalar.copy(osb[:], po.rearrange("p a n -> p (a n)"))
                else:
                    nc.vector.tensor_copy(osb[:], po.rearrange("p a n -> p (a n)"))
                nc.sync.dma_start(out_d[Q * 4 + so], osb[:])

            def emit_outproj_so(Q, so):
                po = psmm.tile([128, 2, 512], F32, tag="mm", name="po")
                for co in range(2):
                    emit_po_co(po, aTs[Q], co, so, stop=(co == 1))
                emit_osb(Q, so, po, "vector")

            def outproj_thunks(Q):
                return [lambda so=so: emit_outproj_so(Q, so) for so in range(4)]

            tail_pos = {}

            # ---- attention ------------------------------------------------
            # pending: deferred PE work (later proj chunks, prior outproj)
            # injected between attention chunks so the PE never stalls on the
            # scalar engine's exp latency.
            emit_proj0()

            for Q in range(4):
                pending = deque()
                if Q > 0:
                    pending.extend(outproj_thunks(Q - 1))
                if Q < 3:
                    pending.extend(proj_thunks(Q + 1))
                # spread injections proportionally over this block's chunks,
                # sparing the first two (the normalize chain's latency and
                # the pipeline refill live there)
                nblk = 8 * (Q + 1)
                npend = len(pending)
                bchunk = 0
                injected = 0
                aT = apool.tile([128, 2, 512], BF16, tag="aT", name="aT")
                aTs[Q] = aT
                for mo in range(2):
                    nchunks = (Q + 1) * 4
                    out_ps = [
                        psout.tile([65, 512], F32, tag="out", name=f"out_ps{_h}")
                        for _h in range(2)
                    ]
                    prevs = []  # [(jc, lo, ex)] awaiting PV; depth 2 hides exp

                    def emit_pv(jc, lo, ex, last):
                        for hp in range(2):
                            h = 2 * mo + hp
                            nc.tensor.matmul(
                                out_ps[hp][:, lo:],
                                vts[jc][:, h * 65:(h + 1) * 65],
                                ex[:, hp, lo:],
                                start=(jc == 0),
                                stop=last,
                                skip_group_check=True,
                            )

                    for jc in range(nchunks):
                        d = jc - 4 * Q if jc // 4 == Q else None
                        lo = 128 * d if d else 0
                        sc = psmm.tile([128, 2, 512], F32, tag="mm", name="sc")
                        for hp in range(2):
                            nc.tensor.matmul(
                                sc[:, hp, lo:],
                                kts[mo][jc // 4][hp * 64:(hp + 1) * 64,
                                                 (jc % 4) * 128:(jc % 4 + 1) * 128],
                                qts[mo][Q][hp * 64:(hp + 1) * 64, lo:],
                                start=True,
                                stop=True,
                                skip_group_check=True,
                            )
                        ex = work.tile([128, 2, 512], BF16, tag="exp", name="ex",
                                       bufs=4)
                        nc.scalar.activation(ex[:, :, lo:], sc[:, :, lo:], EXP,
                                             scale=SCALE)
                        if d is not None:  # causal mask on the diagonal block
                            nc.vector.tensor_mul(
                                ex[:, :, lo:lo + 128], ex[:, :, lo:lo + 128],
                                maskt[:],
                            )
                        if len(prevs) == 2:
                            emit_pv(*prevs.pop(0), last=False)
                        prevs.append((jc, lo, ex))
                        bchunk += 1
                        if bchunk > 2:
                            target = (bchunk - 2) * npend // max(nblk - 2, 1)
                            while injected < target and pending:
                                pending.popleft()()
                                injected += 1
                    while prevs:
                        emit_pv(*prevs.pop(0), last=(not prevs))

                    tail = (Q == 3 and mo == 1)
                    if tail:
                        # keep the PE warm through the final normalize chain
                        # so the last output projection runs at 2.4 GHz
                        emit_warm(20)
                        # the co=0 accumulation passes of the final output
                        # projection only need aT[:, 0, :], which block (3,0)
                        # finished -- run the first pair under the normalize
                        # chain (only 2 psum slots exist, so pairs of two)
                        for so in range(2):
                            tail_pos[so] = psmm.tile([128, 2, 512], F32,
                                                     tag="mm", name="po")
                            emit_po_co(tail_pos[so], aTs[3], 0, so, stop=False)

                    # normalization, entirely off the PE: vector computes the
                    # reciprocals up front, gpsimd broadcasts, vector scales.
                    # Tail fast path: den copy on the (now idle) scalar engine
                    # and the broadcast as a K=1 ones matmul on the PE.
                    rds = []
                    for hp in range(2):
                        den = rpool.tile([1, 512], F32, tag="den", name="den")
                        if tail:
                            nc.scalar.copy(den[:], out_ps[hp][64:65, :])
                        else:
                            nc.vector.tensor_copy(den[:], out_ps[hp][64:65, :])
                        rd = rpool.tile([1, 512], F32, tag="rd", name="rd")
                        # custom-DVE ops read SBUF only on real HW (CoreSim
                        # accepts a psum src but hardware returns garbage)
                        nc.vector.reciprocal_approx_fast(out=rd[:], in_=den[:])
                        rdb = rpool.tile([64, 512], F32, tag="rdb", name="rdb")
                        nc.gpsimd.partition_broadcast(rdb[:], rd[:], channels=64)
                        rds.append(rdb)
                    for hp in range(2):
                        nc.vector.scalar_tensor_tensor(
                            out=aT[hp * 64:(hp + 1) * 64, mo, :],
                            in0=out_ps[hp][0:64, :],
                            scalar=1.0,
                            in1=rds[hp][:],
                            op0=MULT,
                            op1=MULT,
                        )
                while pending:
                    pending.popleft()()

            # tail finale: finish the first out-proj pair (co=1 waits only on
            # the last normalize), then the second pair; casts alternate
            # between the vector and the now-idle scalar engine
            for so in range(2):
                emit_po_co(tail_pos[so], aTs[3], 1, so, stop=True)
                emit_osb(3, so, tail_pos[so], "scalar" if so % 2 else "vector")
            for so in range(2, 4):
                po = psmm.tile([128, 2, 512], F32, tag="mm", name="po")
                emit_po_co(po, aTs[3], 0, so, stop=False)
                emit_po_co(po, aTs[3], 1, so, stop=True)
                emit_osb(3, so, po, "scalar" if so % 2 else "vector")

    nc.compile()
    return nc


_NC = None


def _get_nc():
    global _NC
    if _NC is None:
        _NC = build_bass()
    return _NC


def _causal_mask():
    m = (np.arange(128)[:, None] <= np.arange(128)[None, :]).astype(np.float32)
    return np.broadcast_to(m[:, None, :], (128, 2, 128))


def _bf(a):
    return np.ascontiguousarray(a.astype(ml_dtypes.bfloat16))


def _prep_core_inputs(x, Wq, Wk, Wv, Wo, c):
    b, g = divmod(c, 4)
    cols = slice(g * GC, (g + 1) * GC)
    xb = np.asarray(x[b], np.float32)
    return {
        "xc": _bf(xb.reshape(4, 512, 8, 128).transpose(0, 3, 2, 1)),
        "wq": _bf(Wq.T[:, cols].reshape(8, 128, GC).transpose(1, 0, 2)),
        "wk": _bf(Wk.T[:, cols].reshape(8, 128, GC).transpose(1, 0, 2)),
        "wv": _bf(Wv.T[:, cols].reshape(8, 128, GC).transpose(1, 0, 2)),
        "wo": _bf(Wo[:, cols].T.reshape(2, 128, D).transpose(1, 0, 2)),
        "mask": _bf(_causal_mask()),
    }


def kernel(in_features, Wq, Wk, Wv, Wo):
    global LAST_RESULTS
    nc = _get_nc()

    x = np.asarray(in_features, np.float32)
    Wq = np.asarray(Wq, np.float32)
    Wk = np.asarray(Wk, np.float32)
    Wv = np.asarray(Wv, np.float32)
    Wo = np.asarray(Wo, np.float32)

    in_maps = [_prep_core_inputs(x, Wq, Wk, Wv, Wo, c) for c in range(N_CORES)]

    res = bass_utils.run_bass_kernel_spmd(
        nc, in_maps, core_ids=list(range(N_CORES)), trace=TRACE,
    )
    LAST_RESULTS = res
    parts = [res.results[c]["out"].astype(np.float32).reshape(S, D)
             for c in range(N_CORES)]
    out = np.stack([
        parts[4 * b] + parts[4 * b + 1] + parts[4 * b + 2] + parts[4 * b + 3]
        for b in range(B)
    ]).astype(np.float32)
    return out
